# revision 86
# baseline (speedup 1.0000x reference)
"""Multi-head attention (B=4, N=2048, D=1024, H=16) on 8 Trainium2 cores.

Sharding: core = (batch b, head-group hg) -> 4 batches x 2 groups of 8 heads.

v2 design (all-bf16 matmuls; fp8e4m3 anywhere in the Q/K path or the output
projection measures 2-4e-2 max-rel-err by itself — peaked-softmax queries
and the 5.5-sigma max-statistic over 8.4M outputs defeat 3-mantissa-bit
quantization) with the following performance additions over the 309us
baseline (TimelineSim 302.3us, PE engine ~92% utilized at its bf16 floor):
  - bf16 PE transposes (1.0 cyc/row vs f32's 2.0): O is normalized into
    bf16 before the transpose — bit-identical output, since the ot copy
    rounded to bf16 anyway; the PSUM->SBUF transpose copies also drop to
    the DVE 2-byte fast mode.
  - No dummy zeroing matmuls: the first write of each PSUM accumulation
    group uses start=True, whose pending-zero marking makes every group's
    first touch an overwrite (O accumulators, transpose banks).
  - Split output projection: head-pairs 0-1 of each i-block project into a
    bf16 partial mid-stream (their transposed O blocks finish ~130 steps
    before pairs 2-3), so the end-of-kernel drain only runs the second
    half of the contraction plus a tensor_add.
  - Credit pacing retuned (890 -> 940 cycles/step) and the PV emission
    lag retuned (5 -> 7 steps, refitting the bf16-transpose/split-proj cost
    profile; 8 overruns the 8-deep p-tile pool) for the lighter PE.

Everything else is v2: PV with the P^T block stationary, denominators via
1-col ones-matmuls into a persistent PSUM bank, V bias folded into the
host epilogue, deadline-sorted credit-paced PE fillers, 12 warmup matmuls
for the p-state ramp, host pre-swizzled weights.

PSUM budget (8 banks): s 2x[128,1024]=4, o [128,512]=1, dn [128,512]=1,
y 2x[128,512]=2 (K/Q/V/proj fillers + transposes, rotating).
"""

import sys

if "/opt/trn_rl_repo" not in sys.path:
    sys.path.insert(0, "/opt/trn_rl_repo")

from collections import deque
from contextlib import ExitStack

import numpy as np

B, N, D, H = 4, 2048, 1024, 16
HG = 2                 # head groups (tensor parallel)
NCORES = B * HG        # 8
DH = D // HG           # 512 features per group = 8 heads * 64
HH = H // HG           # 8 heads per core
P = 128
KC = D // P            # 8 contraction chunks over d_model
CP = HH // 2           # 4 head pairs per core
TJ = N // P            # 16 key 128-chunks
IB = 1024              # i-block (exp free-dim)
NI = N // IB           # 2
SCALE = (D // H) ** -0.5

# Schraudolph-to-bf16 exp: int16 pattern = s*TRICK_A + TRICK_B read as bf16
# = exp(s*SCALE)*(1+eps(f)), eps zero-mean, |eps| < 4.2%.
LOG2E = 1.4426950408889634
TRICK_A = 128.0 * LOG2E * SCALE
# 128*127 (bias) - 7.3348 (centers ln((1+f)/2^f)) + 0.5 (round via truncate)
TRICK_B = 16256.0 - 7.3348 + 0.5
# Exp tiles routed to DVE via the bit-trick. Empty: the trick measured
# numerically safe (~9e-3) but the DVE routing never beat the v2 schedule
# in TimelineSim (312.0us vs 309.1us — the ACT relief doesn't convert into
# wall time; the PE filler pacing is the binding constraint).
DVE_EXP_J = frozenset()

_cached = {}


def _build():
    import concourse.mybir as mybir
    import concourse.tile as tile
    from concourse import bacc, masks

    f32 = mybir.dt.float32
    bf16 = mybir.dt.bfloat16
    i16 = mybir.dt.int16
    AF = mybir.ActivationFunctionType
    MUL = mybir.AluOpType.mult
    ADD = mybir.AluOpType.add

    nc = bacc.Bacc("TRN2", target_bir_lowering=False, debug=False,
                   enable_asserts=False)

    # weights arrive pre-swizzled from the host into per-partition-contiguous
    # layouts so every DMA moves >=1KB runs: wq/wk [CP, P, KC, 128] (m-chunk
    # major), wv [HH, P, KC, 64], wp [P, CP, D].
    xt = nc.dram_tensor("xt", (D, N), bf16, kind="ExternalInput").ap()
    wqh = nc.dram_tensor("wqh", (CP, P, KC, P), bf16, kind="ExternalInput").ap()
    wkh = nc.dram_tensor("wkh", (CP, P, KC, P), bf16, kind="ExternalInput").ap()
    wvh = nc.dram_tensor("wvh", (HH, P, KC, 64), bf16, kind="ExternalInput").ap()
    wph = nc.dram_tensor("wph", (P, CP, D), bf16, kind="ExternalInput").ap()
    bqk = nc.dram_tensor("bqk", (1, 2 * DH), f32, kind="ExternalInput").ap()
    y = nc.dram_tensor("y", (N, D), bf16, kind="ExternalOutput").ap()

    xt_r = xt.rearrange("(ko p) t -> p ko t", p=P)

    with tile.TileContext(nc) as tc, ExitStack() as ctx:
        const = ctx.enter_context(tc.tile_pool(name="const", bufs=1))
        persist = ctx.enter_context(tc.tile_pool(name="persist", bufs=1))
        ppool = ctx.enter_context(tc.tile_pool(name="pp", bufs=8))
        ospool = ctx.enter_context(tc.tile_pool(name="osb", bufs=2))
        otpool = ctx.enter_context(tc.tile_pool(name="ot", bufs=2))
        dpool = ctx.enter_context(tc.tile_pool(name="dv", bufs=2))
        ypool = ctx.enter_context(tc.tile_pool(name="yb", bufs=5))
        yhpool = ctx.enter_context(tc.tile_pool(name="yh", bufs=32))
        psp = ctx.enter_context(tc.tile_pool(name="psp", bufs=1, space="PSUM"))

        # ---- consts ----
        wconst = const.tile([P, 512], bf16)
        nc.vector.memset(wconst[:], 0.0)
        ones_bf = const.tile([P, 16], bf16)
        nc.vector.memset(ones_bf[:], 1.0)

        # ---- persistent SBUF ----
        xt_sb = persist.tile([P, KC, N], bf16)
        wk_sb = persist.tile([P, CP, KC, P], bf16)   # [p, m, k, 128]
        wq_sb = persist.tile([P, CP, KC, P], bf16)
        wv_sb = persist.tile([P, HH, KC, 64], bf16)  # [p, h, k, 64]
        wp_sb = persist.tile([P, CP, D], bf16)
        kt = persist.tile([P, CP, N], bf16)        # K^T [feat128(pair), c, keytok]
        qt = persist.tile([P, CP, N], bf16)        # Q^T [feat128(pair), c, qtok]
        vsb = persist.tile([P, TJ, HH, 64], bf16)  # V [keytok128, j, h, feat]
        bqk_sb = const.tile([P, 1, 2 * CP], f32)

        # ---- DMAs (gpsimd queue: cheap issue), prefix-critical first ----
        nc.sync.dma_start(bqk_sb[:], bqk.rearrange("a (mo p) -> p a mo", p=P))
        nc.sync.dma_start(wk_sb[:, 0], wkh[0])
        # x transfers split by contraction half: the K/Q projection's first
        # four k-chunk matmuls start while the second half still streams
        nc.sync.dma_start(xt_sb[:, 0:2, 0:512], xt_r[:, 0:2, 0:512])
        nc.sync.dma_start(xt_sb[:, 2:4, 0:512], xt_r[:, 2:4, 0:512])
        nc.sync.dma_start(xt_sb[:, 4:6, 0:512], xt_r[:, 4:6, 0:512])
        nc.sync.dma_start(xt_sb[:, 6:8, 0:512], xt_r[:, 6:8, 0:512])
        nc.sync.dma_start(wq_sb[:, 0], wqh[0])
        nc.sync.dma_start(xt_sb[:, 0:4, 512:1024], xt_r[:, 0:4, 512:1024])
        nc.sync.dma_start(xt_sb[:, 4:8, 512:1024], xt_r[:, 4:8, 512:1024])
        nc.sync.dma_start(wv_sb[:, 0], wvh[0])
        nc.sync.dma_start(xt_sb[:, :, 1024:1536], xt_r[:, :, 1024:1536])
        nc.sync.dma_start(xt_sb[:, :, 1536:2048], xt_r[:, :, 1536:2048])
        nc.sync.dma_start(wv_sb[:, 1:HH], wvh[1:HH].rearrange("h p k f -> p h k f"))
        nc.sync.dma_start(wk_sb[:, 1:CP], wkh[1:CP].rearrange("m p k f -> p m k f"))
        nc.sync.dma_start(wq_sb[:, 1:CP], wqh[1:CP].rearrange("m p k f -> p m k f"))
        nc.sync.dma_start(wp_sb[:], wph)

        # identity for PE transposes (gpsimd, after the DMA issues).
        # bf16: transposes run at 1.0 cyc/row vs f32's 2.0.
        ident_bf = const.tile([P, P], bf16)
        masks.make_identity(nc, ident_bf[:])

        # preload the exp table while ACT is idle
        dummy = const.tile([1, 16], f32)
        nc.scalar.activation(dummy[:], ones_bf[0:1, :], AF.Exp)

        # persistent PSUM: denominators. Zeroed once (see emit_pv on the
        # zero-region constraint); every denom matmul accumulates.
        dn = psp.tile([P, 512], f32, tag="d", bufs=1, name="dn")
        nc.tensor.matmul(dn[:, 0:NI * HH * 8], wconst[:, 0:P],
                         wconst[:, 0:NI * HH * 8], start=True, stop=False,
                         skip_group_check=True)

        # warmup: absorb the PE p-state ramp before real work dispatches.
        for _ in range(12):
            wm = psp.tile([P, IB], f32, tag="s", bufs=2, name="wm")
            nc.tensor.matmul(wm[:, 0:512], wconst[:, 0:P], wconst[:],
                             start=True, stop=True, skip_group_check=True)

        # ================= emission helpers =================
        def _bufs(tag):
            return 2 if tag in ("s", "y") else 1

        kq_slot = [0]
        v_slot = [0]

        def _kq_psum(tag):
            return psp.tile([P, 512], f32, tag=tag, bufs=_bufs(tag),
                            name="ptkq")[:, 0:256]

        def _v_psum(tag):
            return psp.tile([P, 512], f32, tag=tag, bufs=_bufs(tag),
                            name="ptv")[:, 0:64]

        def emit_k_chunk(m, n, tag, half=None):
            """kt[:, m, n*512+...] — 8 matmuls + bias-add copy."""
            t0, tw = (0, 512) if half is None else (half * 256, 256)
            c0 = n * 512 + t0
            if tw == 512:
                pt = psp.tile([P, 512], f32, tag=tag, bufs=_bufs(tag),
                              name="ptk")
            else:
                pt = _kq_psum(tag)
            for k in range(KC):
                nc.tensor.matmul(pt[:, 0:tw], wk_sb[:, m, k, :],
                                 xt_sb[:, k, c0:c0 + tw],
                                 start=(k == 0), stop=(k == KC - 1))
            nc.vector.tensor_scalar_add(
                kt[:, m, c0:c0 + tw], pt[:, 0:tw],
                bqk_sb[:, 0, CP + m:CP + m + 1])

        def emit_q_chunk(m, n, tag, half=None):
            t0, tw = (0, 512) if half is None else (half * 256, 256)
            c0 = n * 512 + t0
            if tw == 512:
                pt = psp.tile([P, 512], f32, tag=tag, bufs=_bufs(tag),
                              name="ptq")
            else:
                pt = _kq_psum(tag)
            for k in range(KC):
                nc.tensor.matmul(pt[:, 0:tw], wq_sb[:, m, k, :],
                                 xt_sb[:, k, c0:c0 + tw],
                                 start=(k == 0), stop=(k == KC - 1))
            nc.vector.tensor_scalar_add(
                qt[:, m, c0:c0 + tw], pt[:, 0:tw],
                bqk_sb[:, 0, m:m + 1])

        def emit_v_slice(h, g, tag):
            """vsb[:, g, h, :] — V features of head h for keytok chunk g."""
            n, tt = divmod(g, 4)
            pv = _v_psum(tag)
            for k in range(KC):
                nc.tensor.matmul(
                    pv[:],
                    xt_sb[:, k, n * 512 + tt * P:n * 512 + (tt + 1) * P],
                    wv_sb[:, h, k, :],
                    start=(k == 0), stop=(k == KC - 1))
            nc.vector.tensor_copy(vsb[:, g, h, :], pv[:])

        ot_tiles = {}

        yhalf = {}   # (i, t, o) -> bf16 partial (head-pairs 0-1)

        def emit_proj_a(i, t, o, tag):
            """First half of the projection contraction (pairs 0,1): these
            ot blocks finish mid-stream, so this work leaves the tail."""
            ot_i = ot_tiles[i]
            yp = psp.tile([P, 512], f32, tag=tag, bufs=_bufs(tag), name="yp")
            for cc in range(2):
                nc.tensor.matmul(yp[:], ot_i[:, cc, t * P:(t + 1) * P],
                                 wp_sb[:, cc, o * 512:(o + 1) * 512],
                                 start=(cc == 0), stop=(cc == 1))
            ph = yhpool.tile([P, 512], bf16, tag="yh", name="yh")
            nc.vector.tensor_copy(ph[:], yp[:])
            yhalf[(i, t, o)] = ph

        def emit_proj(i, t, o, tag):
            """Remaining pairs (+ the stored partial when one exists). The
            partial merge alternates between a DVE tensor_add and a PE
            identity-matmul accumulate + ACT copy, so the tail drain is not
            paced by a single engine's serial add chain."""
            ot_i = ot_tiles[i]
            c0 = 2 if (i, t, o) in yhalf else 0
            on_act = False   # measured: all-DVE adds beat mixed PE/ACT merge
            yp = psp.tile([P, 512], f32, tag=tag, bufs=_bufs(tag), name="yp")
            for cc in range(c0, CP):
                nc.tensor.matmul(yp[:], ot_i[:, cc, t * P:(t + 1) * P],
                                 wp_sb[:, cc, o * 512:(o + 1) * 512],
                                 start=(cc == c0),
                                 stop=(cc == CP - 1 and not on_act))
            ysb = ypool.tile([P, 512], bf16, tag="ysb", name="ysb")
            if c0 == 2 and on_act:
                # yp += I.T @ partial (exact bf16 pass-through into f32 psum)
                nc.tensor.matmul(yp[:], ident_bf[:],
                                 yhalf.pop((i, t, o))[:],
                                 start=False, stop=True)
                nc.scalar.activation(ysb[:], yp[:], AF.Copy)
            elif c0 == 2:
                nc.vector.tensor_add(ysb[:], yp[:], yhalf.pop((i, t, o))[:])
            else:
                nc.vector.tensor_copy(ysb[:], yp[:])
            r0 = i * IB + t * P
            nc.sync.dma_start(y[r0:r0 + P, o * 512:(o + 1) * 512], ysb[:])

        # filler queue: (deadline_step, rows, fn(tag)) in deadline order.
        fillers = deque()

        def F(rows, fn, deadline=10**9):
            fillers.append((deadline, rows, fn))

        def pump(credit, step=-1):
            while fillers and (fillers[0][0] <= step
                               or fillers[0][1] <= credit):
                _, rows, fn = fillers.popleft()
                fn("y")
                credit -= rows
            return credit

        # ================= prefix =================
        emit_k_chunk(0, 0, "s")
        emit_q_chunk(0, 0, "s")
        emit_q_chunk(0, 1, "s")
        emit_v_slice(0, 0, "s")
        emit_v_slice(0, 1, "s")

        NU = NI * HH                      # 16 units
        seq = [(0, 0), (0, 1), (1, 0), (1, 1),
               (0, 2), (0, 3), (1, 2), (1, 3),
               (0, 4), (0, 5), (0, 6), (0, 7),
               (1, 4), (1, 5), (1, 6), (1, 7)]
        pos_of = {u: p for p, u in enumerate(seq)}

        # ---- filler queue: deadline-sorted fine-grained items ----
        events = []   # (deadline_step, rows, fn)
        for h in range(HH):
            for g in range(TJ):
                if h == 0 and g < 2:
                    continue   # prefix
                events.append((16 * pos_of[(0, h)] + g, 512,
                               lambda tag, h=h, g=g: emit_v_slice(h, g, tag)))
        for c in range(CP):
            p0 = 16 * pos_of[(0, 2 * c)]
            p1 = 16 * pos_of[(1, 2 * c)]
            for n in range(4):
                for half in range(2):
                    if c == 0 and n == 0:
                        continue   # prefix
                    events.append((max(0, p0 + 4 * n - 2), 2048,
                                   lambda tag, c=c, n=n, hf=half:
                                   emit_k_chunk(c, n, tag, hf)))
            for n in range(4):
                for half in range(2):
                    if c == 0 and n < 2:
                        continue   # prefix
                    dl = p0 - 2 if n < 2 else p1 - 2
                    events.append((max(0, dl), 2048,
                                   lambda tag, c=c, n=n, hf=half:
                                   emit_q_chunk(c, n, tag, hf)))
        events.sort(key=lambda e: e[0])
        for dl, rows, fn in events:
            F(rows, fn, deadline=dl)

        # ================= attention units =================
        p_of = {}      # (u, j) -> p tile
        o_ps_of = {}   # u -> O psum accumulator
        osb_of = {}    # (i, c) -> normalized-O sbuf tile

        def emit_scores_exp(u, j):
            i, h = seq[u]
            c, hp = divmod(h, 2)
            r0, r1 = hp * 64, hp * 64 + 64
            s = psp.tile([P, IB], f32, tag="s", bufs=2, name="s")
            for iq in range(2):
                nc.tensor.matmul(
                    s[:, iq * 512:(iq + 1) * 512],
                    kt[r0:r1, c, j * P:(j + 1) * P],
                    qt[r0:r1, c, i * IB + iq * 512:i * IB + (iq + 1) * 512],
                    start=True, stop=True)
            p = ppool.tile([P, IB], bf16, tag="p", name="p")
            if j in DVE_EXP_J:
                # Schraudolph bit-trick on the (otherwise idle) DVE: int16
                # pattern -> bf16 exp; frees ~30% of the saturated ACT stream
                nc.vector.tensor_scalar(p[:].bitcast(i16), s[:],
                                        TRICK_A, TRICK_B, op0=MUL, op1=ADD)
            else:
                nc.scalar.activation(p[:], s[:], AF.Exp, scale=SCALE)
            p_of[(u, j)] = p

        def emit_pv(u, j):
            i, h = seq[u]
            if j == 0:
                o_ps_of[u] = psp.tile([P, 512], f32, tag="o", bufs=1,
                                      name="ops")
            o_ps = o_ps_of[u]
            p = p_of.pop((u, j))
            for t in range(8):
                # first write's start=True pending-zeroes the whole bank (so
                # every group's first touch is an overwrite); last stops the
                # group so the next unit's start finds it clear — replaces
                # the v2 dummy zeroing matmul.
                first = j == 0 and t == 0
                last = j == TJ - 1 and t == 7
                nc.tensor.matmul(o_ps[:, t * 64:(t + 1) * 64],
                                 p[:, t * P:(t + 1) * P], vsb[:, j, h, :],
                                 start=first, stop=last,
                                 skip_group_check=not (first or last))
            for t in range(8):
                nc.tensor.matmul(dn[:, u * 8 + t:u * 8 + t + 1],
                                 p[:, t * P:(t + 1) * P], ones_bf[:, 0:1],
                                 start=False, stop=False,
                                 skip_group_check=True)

        def emit_norm(u):
            """Normalize O of unit u into osb (per-partition 1/denom)."""
            i, h = seq[u]
            c, hp = divmod(h, 2)
            o_ps = o_ps_of.pop(u)
            rcp = dpool.tile([P, 8], f32, tag="rcp", name="rcp")
            nc.vector.reciprocal(rcp[:], dn[:, u * 8:u * 8 + 8])
            if hp == 0:
                osb_of[(i, c)] = ospool.tile([P, 8, 2, 64], bf16, tag="osb",
                                             name="osb")
            osb = osb_of[(i, c)]
            nc.vector.tensor_mul(
                osb[:, :, hp, :],
                o_ps[:].rearrange("p (t f) -> p t f", f=64),
                rcp[:, :, None].broadcast_to([P, 8, 64]))

        def emit_transposes(i, c):
            """O pair-block [qtok, 128feat] -> ot [128feat, qtok] via bf16 PE
            transposes into a bf16 view of the psum bank; the first quarter's
            start=True pending-zeroes the bank (no dummy zeroing matmul) and
            the copy runs in the DVE 2-byte fast mode."""
            osb = osb_of.pop((i, c))
            for g in range(2):
                yslot = psp.tile([P, 512], f32, tag="y", bufs=2, name="tp")
                for tt in range(4):
                    t = g * 4 + tt
                    nc.tensor.matmul(
                        yslot[:, tt * 64:(tt + 1) * 64].bitcast(bf16),
                        osb[:, t, :, :].rearrange("p a b -> p (a b)"),
                        ident_bf[:], is_transpose=True,
                        start=(tt == 0), stop=(tt == 3),
                        skip_group_check=tt not in (0, 3))
                nc.vector.tensor_copy(
                    ot_tiles[i][:, c, g * 512:(g + 1) * 512],
                    yslot[:, 0:256].bitcast(bf16))

        CREDIT_PER_STEP = 940
        CREDIT_CAP = 4200
        PVLAG = 7
        credit = -3000    # delay the first credit pops past the prefix chain
        pending = {}
        tp_done = {0: 0, 1: 0}
        for g in range(NU * TJ + PVLAG + 4):
            credit = pump(credit, g)   # deadline-forced pops
            if g < NU * TJ:
                u, j = divmod(g, TJ)
                if j == 0:
                    i, h = seq[u]
                    if h == 0 and i not in ot_tiles:
                        ot_tiles[i] = otpool.tile([P, CP, IB], bf16,
                                                  tag="ot", name="ot")
                emit_scores_exp(u, j)
            gp = g - PVLAG
            if 0 <= gp < NU * TJ:
                up, jp = divmod(gp, TJ)
                if jp == 0:
                    pass                      # deferred: paired with j1
                elif jp == 1:
                    emit_pv(up, 0)
                    emit_pv(up, 1)
                else:
                    emit_pv(up, jp)
                if jp == TJ - 1:
                    iup, hup = seq[up]
                    emit_norm(up)
                    if hup % 2 == 1:
                        pending.setdefault(g + 6, []).append(
                            ("tp", iup, hup // 2))
            for kind, a1, a2 in pending.pop(g, []):
                emit_transposes(a1, a2)
                tp_done[a1] += 1
                if tp_done[a1] == 2 and a1 == 1:
                    for t in range(8):
                        for o in range(2):
                            F(1024, lambda tag, ii=a1, tt=t, oo=o:
                              emit_proj_a(ii, tt, oo, tag))
                if tp_done[a1] == CP:
                    for t in range(8):
                        for o in range(2):
                            F(2048 if a1 == 0 else 1024,
                              lambda tag, ii=a1, tt=t, oo=o:
                              emit_proj(ii, tt, oo, tag))
            credit = min(credit + CREDIT_PER_STEP, CREDIT_CAP)
            credit = pump(credit, g)
        for gq in sorted(pending):
            for kind, a1, a2 in pending[gq]:
                emit_transposes(a1, a2)
                tp_done[a1] += 1
                if tp_done[a1] == 2 and a1 == 1:
                    for t in range(8):
                        for o in range(2):
                            F(1024, lambda tag, ii=a1, tt=t, oo=o:
                              emit_proj_a(ii, tt, oo, tag))
                if tp_done[a1] == CP:
                    for t in range(8):
                        for o in range(2):
                            F(2048 if a1 == 0 else 1024,
                              lambda tag, ii=a1, tt=t, oo=o:
                              emit_proj(ii, tt, oo, tag))

        # tail: drain remaining fillers at full rate (s banks free now)
        tags = ("s", "s", "y", "y")
        k = 0
        while fillers:
            _, _, fn = fillers.popleft()
            fn(tags[k % 4])
            k += 1

    nc.compile()
    return nc


def _get_nc():
    if "nc" not in _cached:
        _cached["nc"] = _build()
    return _cached["nc"]


def kernel(x, W_qkv, b_qkv, W_proj, b_proj):
    import ml_dtypes
    from concourse.bass_utils import run_bass_kernel_spmd

    bf16 = ml_dtypes.bfloat16
    x = np.asarray(x, dtype=np.float32)
    W_qkv = np.asarray(W_qkv, dtype=np.float32)
    b_qkv = np.asarray(b_qkv, dtype=np.float32)
    W_proj = np.asarray(W_proj, dtype=np.float32)
    b_proj = np.asarray(b_proj, dtype=np.float32)

    in_maps = []
    for core in range(NCORES):
        b, hg = divmod(core, HG)
        qs = slice(DH * hg, DH * (hg + 1))
        ks = slice(D + DH * hg, D + DH * (hg + 1))
        vs = slice(2 * D + DH * hg, 2 * D + DH * (hg + 1))

        def swz_qk(w):   # [D, 512] -> [CP, P, KC, P] (m-major, p-contig)
            return np.ascontiguousarray(
                w.reshape(KC, P, CP, P).transpose(2, 1, 0, 3)).astype(bf16)

        def swz_v(w):    # [D, 512] -> [HH, P, KC, 64]
            return np.ascontiguousarray(
                w.reshape(KC, P, HH, 64).transpose(2, 1, 0, 3)).astype(bf16)

        in_maps.append({
            "xt": np.ascontiguousarray(x[b].T).astype(bf16),
            "wqh": swz_qk(W_qkv[:, qs]),
            "wkh": swz_qk(W_qkv[:, ks]),
            "wvh": swz_v(W_qkv[:, vs]),
            "wph": np.ascontiguousarray(
                W_proj[DH * hg:DH * (hg + 1), :].reshape(CP, P, D)
                .transpose(1, 0, 2)).astype(bf16),
            "bqk": np.concatenate([b_qkv[qs], b_qkv[ks]])[None, :],
        })

    nc = _get_nc()
    res = run_bass_kernel_spmd(nc, in_maps, core_ids=list(range(NCORES)))
    beff = (b_proj.astype(np.float64)
            + b_qkv[2 * D:].astype(np.float64) @ W_proj.astype(np.float64)
            ).astype(np.float32)
    out = np.empty((B, N, D), dtype=np.float32)
    for b in range(B):
        out[b] = (res.results[2 * b]["y"].astype(np.float32)
                  + res.results[2 * b + 1]["y"].astype(np.float32) + beff)
    return out
